# revision 1
# baseline (speedup 1.0000x reference)
"""AVMambaClassifier kernel — data-parallel over batch across 8 NeuronCores.

Shards B=32 into 4 per core, replicates the (<10MB) parameters, runs the
full forward (masked temporal instance norm -> proj -> 4 prenorm Mamba
blocks -> masked attention pool -> linear head) per shard, gathers [32,8].
"""

import numpy as np
import jax
import jax.numpy as jnp
from functools import partial

B, T = 32, 512
DA, DV = 1280, 768
D = DA + DV          # 2048
DM = 256
L = 4
DI = 2 * DM          # 512
DS = 16
DC = 4
R = DM // 16         # 16
H = 128
NC = 8
EPS_TIN = 1e-5
EPS_LN = 1e-5

NCORES = 8
BL = B // NCORES     # 4 per core

_WNAMES = [
    'tin_gamma', 'tin_beta', 'proj_w', 'proj_b', 'ln_g', 'ln_b', 'in_proj_w',
    'conv_w', 'conv_b', 'x_proj_w', 'dt_proj_w', 'dt_proj_b', 'A_log',
    'D_param', 'out_proj_w', 'out_ln_g', 'out_ln_b', 'res_scale', 'attn_w1',
    'attn_b1', 'attn_w2', 'head_w', 'head_b',
]


def _ln(x, g, b):
    mu = x.mean(-1, keepdims=True)
    v = ((x - mu) ** 2).mean(-1, keepdims=True)
    return (x - mu) / jnp.sqrt(v + EPS_LN) * g + b


def _mamba(u, in_w, cw, cb, xw, dtw, dtb, A_log, Dp, ow):
    xz = jnp.einsum('btd,ed->bte', u, in_w)
    x, z = xz[..., :DI], xz[..., DI:]
    xp = jnp.pad(x, ((0, 0), (DC - 1, 0), (0, 0)))
    xc = sum(xp[:, k:k + u.shape[1], :] * cw[:, k] for k in range(DC)) + cb
    x = xc * jax.nn.sigmoid(xc)
    dbc = jnp.einsum('btd,ed->bte', x, xw)
    # softplus(z) = -log(sigmoid(-z)); the direct log1p(exp) form hits a
    # broken activation-lowering set in this neuronx-cc build.
    zdt = jnp.einsum('btr,dr->btd', dbc[..., :R], dtw) + dtb
    dt = -jnp.log(jax.nn.sigmoid(-zdt))
    Bm = dbc[..., R:R + DS]
    Cm = dbc[..., R + DS:]
    A = -jnp.exp(A_log)

    # E[t] = exp(dt_t * A), F[t] = dt_t * B_t * x_t ; h_t = E h_{t-1} + F
    E = jnp.exp(dt[..., None] * A)                       # [B,T,DI,DS]
    F = (dt * x)[..., None] * Bm[:, :, None, :]          # [B,T,DI,DS]

    def comb(a, b):
        (Ea, Fa), (Eb, Fb) = a, b
        return Ea * Eb, Fa * Eb + Fb

    Ec, Fc = jax.lax.associative_scan(comb, (E, F), axis=1)
    ys = jnp.einsum('btds,bts->btd', Fc, Cm)
    y = ys + Dp * x
    y = y * (z * jax.nn.sigmoid(z))
    return jnp.einsum('bti,oi->bto', y, ow)


def _forward(x, lengths, tin_gamma, tin_beta, proj_w, proj_b, ln_g, ln_b,
             in_proj_w, conv_w, conv_b, x_proj_w, dt_proj_w, dt_proj_b,
             A_log, D_param, out_proj_w, out_ln_g, out_ln_b, res_scale,
             attn_w1, attn_b1, attn_w2, head_w, head_b):
    mask = jnp.arange(T)[None, :] < lengths[:, None]
    m = mask[..., None].astype(x.dtype)
    denom = jnp.maximum(lengths, 1).astype(x.dtype)[:, None, None]
    mean = (x * m).sum(1, keepdims=True) / denom
    var = (((x - mean) ** 2) * m).sum(1, keepdims=True) / denom
    xn = (x - mean) / jnp.sqrt(var + EPS_TIN) * m
    xn = (xn * tin_gamma + tin_beta) * m
    h = jnp.einsum('btd,md->btm', xn, proj_w) + proj_b
    for l in range(L):
        y = _mamba(_ln(h, ln_g[l], ln_b[l]), in_proj_w[l], conv_w[l],
                   conv_b[l], x_proj_w[l], dt_proj_w[l], dt_proj_b[l],
                   A_log[l], D_param[l], out_proj_w[l])
        h = h + res_scale * y
    h = _ln(h, out_ln_g, out_ln_b)
    s = jnp.einsum('bth,oh->bto',
                   jnp.tanh(jnp.einsum('btd,hd->bth', h, attn_w1) + attn_b1),
                   attn_w2)[..., 0]
    logits = jnp.where(mask, s, jnp.float32(-1e30))
    e = jnp.exp(logits - logits.max(1, keepdims=True))
    alpha = (e / e.sum(1, keepdims=True)) * mask
    alpha = alpha / jnp.clip(alpha.sum(1, keepdims=True), 1e-9)
    ctx = (alpha[..., None] * h).sum(1)
    return jnp.einsum('bd,cd->bc', ctx, head_w) + head_b


_pfwd = None


def _get_pfwd():
    global _pfwd
    if _pfwd is None:
        _pfwd = jax.pmap(_forward, axis_name='i',
                         in_axes=(0, 0) + (None,) * len(_WNAMES),
                         devices=jax.devices()[:NCORES])
    return _pfwd


def kernel(**inputs) -> np.ndarray:
    x = np.asarray(inputs['x'], np.float32).reshape(NCORES, BL, T, D)
    lengths = np.asarray(inputs['lengths']).astype(np.int32)
    lengths = lengths.reshape(NCORES, BL)
    ws = [jnp.asarray(np.asarray(inputs[n], np.float32)) for n in _WNAMES]
    out = _get_pfwd()(x, lengths, *ws)
    return np.asarray(out, np.float32).reshape(B, NC)


if __name__ == '__main__':
    rng = np.random.default_rng(0)
    ins = {'x': rng.standard_normal((B, T, D), dtype=np.float32),
           'lengths': rng.integers(T // 4, T + 1, size=(B,)).astype(np.int64)}
    # weights: smoke shapes only
    print(kernel(**ins, **{n: np.zeros(1, np.float32) for n in _WNAMES}).shape)

